# revision 4
# baseline (speedup 1.0000x reference)
"""BiRWKV attention Trainium2 kernel, v3.

Full-input contract: kernel(**inputs) takes the complete (unsharded) arrays
    r, k, v : [B=4, T=4096, C=1280] f32
    w, u    : [1, 1, 1280] f32
and returns y [4, 4096, 1280] f32.

Sharding: 8 cores = batch(4) x channel-half(2); WKV is independent per
(batch, channel) so no communication.

Math (per channel, d = exp(-exp(w)), ek = e^k, x = ek*v):
  num[t] = sum_{j<t} d^{t-1-j} x[j] + e^u x[t] + sum_{j>t} d^{j-1-t} x[j]
  den[t] = same with x -> ek;   y = sigmoid(r) * num/den
With INCLUSIVE scans yf[t] = d yf[t-1] + x[t], z[t] = d z[t+1] + x[t]:
  num[t] = c1*yf[t-1] + c2*yf[t] + z[t+1],  c1 = 1 - e^u d, c2 = e^u.
The division and gate are computed in the exponent domain (no divide ALU on
TRN2, reciprocal only on the busy DVE):
  y = num * exp(-(ln(den) + ln(1 + e^{-r})))      [= num/den * sigmoid(r)]
using only Exp/Ln activations, which share one ACT table (a manual
LoadActFuncSet pins it; the greedy table chooser would otherwise thrash
1283ns reloads between the ln-only and exp-only tables).

Device mapping (per core: [T=4096, C_loc=640], channels on partitions in 5
groups of 128, time on the free dim):
  * inputs host-cast fp16, loaded via DMA xbar transpose into [C,T] tiles
  * all 4 scans on DVE, one whole-group [128,4096] instruction each (no
    chunk chaining, no halo; DVE is the only engine with scan support)
  * combine: per 1024-chunk, 3 accumulating matmuls per 512-slice into PSUM
    (ident, diag(c1), diag(c2)), weight-grouped to 3 ldweights/chunk
  * epilogue per chunk: ACT Ln(DEN) -> f32, Pool adds the per-group
    LU = ln(1+e^{-r}) [f32], ACT Exp(-.) -> fp16, DVE multiplies NUM(PSUM)
    by it (fp16 out), DMA store.
  * y is stored transposed [C_loc, T] fp16; host transposes back
"""

import os
import sys
from contextlib import ExitStack

import numpy as np

for _p in ("/opt/trn_rl_repo",):
    if _p not in sys.path and os.path.isdir(_p):
        sys.path.insert(0, _p)

import concourse.bass as bass
import concourse.bacc as bacc
import concourse.tile as tile
from concourse import mybir

# ----------------------------------------------------------------- config
B, T, C = 4, 4096, 1280
N_CORES = 8
C_LOC = C // 2          # 640 channels per core
P = 128                 # partitions
G = C_LOC // P          # 5 channel groups
L = 1024                # matmul/epilogue chunk length
MM = 512                # matmul slice (PSUM bank)
SCAN_DT = mybir.dt.float16
F32 = mybir.dt.float32


def build_nc(t_dim=T, c_loc=C_LOC, chunk=L, halo=0, scan_dt=SCAN_DT,
             body_reps=1):
    """Emit the per-core Bass program (SPMD: all 8 cores run this)."""
    g_cnt = c_loc // P
    nch = t_dim // chunk
    assert c_loc % P == 0 and t_dim % chunk == 0 and chunk % MM == 0

    nc = bacc.Bacc()
    kp = nc.declare_dram_parameter("k", [t_dim, c_loc], scan_dt, isOutput=False)
    vp = nc.declare_dram_parameter("v", [t_dim, c_loc], scan_dt, isOutput=False)
    rp = nc.declare_dram_parameter("r", [t_dim, c_loc], scan_dt, isOutput=False)
    yp = nc.declare_dram_parameter("y", [c_loc, t_dim], scan_dt, isOutput=True)
    dcp = nc.declare_dram_parameter("dcol", [g_cnt, P], F32, isOutput=False)
    # diag(c1) | diag(c2) per group, plus ident, packed as one [P, .] blob
    dgp = nc.declare_dram_parameter("diagc", [P, (2 * g_cnt + 1) * P], scan_dt,
                                    isOutput=False)

    MUL, ADD = mybir.AluOpType.mult, mybir.AluOpType.add
    EXP = mybir.ActivationFunctionType.Exp
    LN = mybir.ActivationFunctionType.Ln
    CPY = mybir.ActivationFunctionType.Copy

    # the one ACT table serving every func this kernel uses (Exp, Ln, Copy)
    from concourse.hw_specs import get_activation_tables
    _tabs = list(get_activation_tables(nc.m.arch).items())
    LNEXP_ID = next(i for i, (_, s) in enumerate(_tabs)
                    if EXP in s and LN in s)

    with tile.TileContext(nc) as tc, ExitStack() as ctx:
        pers = ctx.enter_context(tc.tile_pool(name="pers", bufs=1))
        grp = ctx.enter_context(tc.tile_pool(name="grp", bufs=2))
        chk = ctx.enter_context(tc.tile_pool(name="chk", bufs=3))
        psum = ctx.enter_context(tc.tile_pool(name="psum", bufs=2,
                                              space="PSUM"))

        DGI = pers.tile([P, (2 * g_cnt + 1) * P], scan_dt, tag="dgi",
                        name="DGI")
        DCOL = pers.tile([P, g_cnt], F32, tag="dcol", name="DCOL")
        nc.sync.dma_start(out=DGI, in_=dgp[:, :])
        nc.sync.dma_start(out=DCOL, in_=dcp.rearrange("g p -> p g"))
        ident = DGI[:, 2 * g_cnt * P: (2 * g_cnt + 1) * P]

        # pin the ln+exp ACT table once, up front
        nc.scalar.add_instruction(mybir.InstLoadActFuncSet(
            name=nc.get_next_instruction_name(), act_func_set_id=LNEXP_ID,
            ins=[], outs=[]))

        def dg1(g):
            return DGI[:, 2 * g * P: (2 * g + 1) * P]

        def dg2(g):
            return DGI[:, (2 * g + 1) * P: (2 * g + 2) * P]

        def dbc(g, ncols):  # broadcast the per-channel decay column
            t = DCOL[:, g:g + 1]
            return bass.AP(tensor=t.tensor, offset=t.offset,
                           ap=[t.ap[0], [0, ncols]])

        for _rep in range(body_reps):
            state = {}      # g -> (EK, EKV, SP, YA, YB, ZA, ZB)
            pend = None     # (g, n, NUM, DEN) awaiting epilogue

            def preload(g):
                """DMA loads + per-group elementwise prep for group g."""
                c0 = g * P
                KT = grp.tile([P, t_dim], scan_dt, tag="kt", name=f"kt{g}")
                VT = grp.tile([P, t_dim], scan_dt, tag="vt", name=f"vt{g}")
                RT = grp.tile([P, t_dim], scan_dt, tag="rt", name=f"rt{g}")
                LUF = grp.tile([P, t_dim], F32, tag="luf", name=f"luf{g}")
                YA = grp.tile([P, t_dim + 1], scan_dt, tag="ya", name=f"ya{g}")
                YB = grp.tile([P, t_dim + 1], scan_dt, tag="yb", name=f"yb{g}")
                ZA = grp.tile([P, t_dim], scan_dt, tag="za", name=f"za{g}")
                ZB = grp.tile([P, t_dim], scan_dt, tag="zb", name=f"zb{g}")
                nc.sync.dma_start(out=KT, in_=kp[:, c0:c0 + P], transpose=True)
                nc.sync.dma_start(out=VT, in_=vp[:, c0:c0 + P], transpose=True)
                nc.sync.dma_start(out=RT, in_=rp[:, c0:c0 + P], transpose=True)
                # EK = e^k (in place); EKV = EK*v (in place, Pool);
                # LU = ln(1 + e^{-r})
                nc.scalar.activation(out=KT, in_=KT, func=EXP)
                nc.scalar.activation(out=RT, in_=RT, func=EXP, scale=-1.0)
                nc.scalar.activation(out=LUF, in_=RT, func=LN, bias=1.0)
                nc.gpsimd.tensor_tensor(out=VT, in0=KT, in1=VT, op=MUL)
                nc.vector.memset(YA[:, 0:1], 0.0)
                nc.vector.memset(YB[:, 0:1], 0.0)
                nc.vector.memset(ZA[:, t_dim - 1:t_dim], 0.0)
                nc.vector.memset(ZB[:, t_dim - 1:t_dim], 0.0)
                state[g] = (KT, VT, LUF, YA, YB, ZA, ZB)

            def scans(g, half):
                """Scans for group g over time-half `half` (0=low, 1=high).
                YA[:, 1+t] = yf[t];  ZA[:, j] = z[j+1] (ZA[:,T-1] = 0).
                Fwd runs low-half first; bwd runs high-half first; both
                chain exactly through the boundary column."""
                EK, EKV, SP_, YA, YB, ZA, ZB = state[g]
                H = t_dim // 2
                if half == 0:  # fwd low, bwd high
                    for Y, X in ((YA, EKV), (YB, EK)):
                        nc.vector.tensor_tensor_scan(
                            out=Y[:, 1:1 + H], data0=dbc(g, H),
                            data1=X[:, 0:H],
                            initial=Y[:, 0:1], op0=MUL, op1=ADD)
                    for Z, X in ((ZA, EKV), (ZB, EK)):
                        nc.vector.tensor_tensor_scan(
                            out=Z[:, H - 1:t_dim - 1][:, ::-1],
                            data0=dbc(g, H),
                            data1=X[:, H:t_dim][:, ::-1],
                            initial=0.0, op0=MUL, op1=ADD)
                else:  # fwd high, bwd low
                    for Y, X in ((YA, EKV), (YB, EK)):
                        nc.vector.tensor_tensor_scan(
                            out=Y[:, 1 + H:1 + t_dim], data0=dbc(g, H),
                            data1=X[:, H:t_dim],
                            initial=Y[:, H:H + 1], op0=MUL, op1=ADD)
                    for Z, X in ((ZA, EKV), (ZB, EK)):
                        nc.vector.tensor_tensor_scan(
                            out=Z[:, 0:H - 1][:, ::-1],
                            data0=dbc(g, H - 1),
                            data1=X[:, 1:H][:, ::-1],
                            initial=Z[:, H - 1:H], op0=MUL, op1=ADD)

            def body(g, n, slot):
                """Combine matmuls for chunk (g, n) -> PSUM NUM/DEN."""
                EK, EKV, SP_, YA, YB, ZA, ZB = state[g]
                t0 = n * chunk
                NUM = psum.tile([P, chunk], F32, tag="num", name=f"num{slot}")
                DEN = psum.tile([P, chunk], F32, tag="den", name=f"den{slot}")
                sl = [(s, s + MM) for s in range(0, chunk, MM)]
                for a, b in sl:
                    nc.tensor.matmul(NUM[:, a:b], ident, ZA[:, t0 + a:t0 + b],
                                     start=True, stop=False)
                    nc.tensor.matmul(DEN[:, a:b], ident, ZB[:, t0 + a:t0 + b],
                                     start=True, stop=False)
                for a, b in sl:
                    nc.tensor.matmul(NUM[:, a:b], dg1(g),
                                     YA[:, t0 + a: t0 + b],
                                     start=False, stop=False)
                    nc.tensor.matmul(DEN[:, a:b], dg1(g),
                                     YB[:, t0 + a: t0 + b],
                                     start=False, stop=False)
                for a, b in sl:
                    nc.tensor.matmul(NUM[:, a:b], dg2(g),
                                     YA[:, 1 + t0 + a: 1 + t0 + b],
                                     start=False, stop=True)
                    nc.tensor.matmul(DEN[:, a:b], dg2(g),
                                     YB[:, 1 + t0 + a: 1 + t0 + b],
                                     start=False, stop=True)
                return NUM, DEN

            def epilogue(g, n, NUM, DEN, slot):
                """y = NUM * exp(-(ln(DEN) + ln(1+e^{-r}))); store.
                DVE-free: ACT stages both PSUM reads, Pool multiplies."""
                LUF = state[g][2]
                t0 = n * chunk
                LD = chk.tile([P, chunk], F32, tag="ld", name=f"ld{slot}")
                NS = chk.tile([P, chunk], scan_dt, tag="ns", name=f"ns{slot}")
                RD = chk.tile([P, chunk], scan_dt, tag="rd", name=f"rd{slot}")
                YT = chk.tile([P, chunk], scan_dt, tag="yt", name=f"yt{slot}")
                nc.scalar.activation(out=LD, in_=DEN, func=LN)
                nc.gpsimd.tensor_tensor(out=LD, in0=LD,
                                        in1=LUF[:, t0:t0 + chunk], op=ADD)
                nc.scalar.activation(out=RD, in_=LD, func=EXP, scale=-1.0)
                nc.scalar.activation(out=NS, in_=NUM, func=CPY)
                nc.gpsimd.tensor_tensor(out=YT, in0=NS, in1=RD, op=MUL)
                nc.sync.dma_start(out=yp[g * P:(g + 1) * P, t0:t0 + chunk],
                                  in_=YT)

            # chunk order [2,3,0,1]: fwd scans fill the low half first while
            # bwd scans fill the high half, so high-half chunks unblock after
            # 6 of the 8 half-scans and low-half chunks after all 8.
            n_order = [n for n in range(nch // 2, nch)] + \
                      [n for n in range(nch // 2)]
            preload(0)
            for g in range(g_cnt):
                if g + 1 < g_cnt:
                    preload(g + 1)
                scans(g, 0)
                scans(g, 1)
                for i, n in enumerate(n_order):
                    num, den = body(g, n, g * nch + i)
                    if pend is not None:
                        epilogue(*pend, g * nch + i)
                    pend = (g, n, num, den)
            epilogue(*pend, g_cnt * nch)
    nc.compile()
    return nc


# ----------------------------------------------------------------- host side
def _derived(w_half, u_half, scan_np_dt):
    """Per-channel-half constant arrays shipped to the device."""
    w64 = w_half.astype(np.float64)
    u64 = u_half.astype(np.float64)
    d = np.exp(-np.exp(w64))                      # decay, in (0,1)
    c1 = 1.0 - np.exp(u64) * d
    c2 = np.exp(u64)
    blob = np.zeros((P, (2 * G + 1) * P), np.float64)
    for g in range(G):
        np.fill_diagonal(blob[:, 2 * g * P:(2 * g + 1) * P],
                         c1.reshape(G, P)[g])
        np.fill_diagonal(blob[:, (2 * g + 1) * P:(2 * g + 2) * P],
                         c2.reshape(G, P)[g])
    np.fill_diagonal(blob[:, 2 * G * P:(2 * G + 1) * P], 1.0)
    return {
        "dcol": np.ascontiguousarray(d.reshape(G, P).astype(np.float32)),
        "diagc": blob.astype(scan_np_dt),
    }


_NC_CACHE = {}


def _get_nc():
    key = (T, C_LOC, L, str(SCAN_DT))
    if key not in _NC_CACHE:
        _NC_CACHE[key] = build_nc(T, C_LOC, L)
    return _NC_CACHE[key]


def _make_in_maps(r, k, v, w, u):
    scan_np_dt = mybir.dt.np(SCAN_DT)
    wf = np.asarray(w).reshape(-1).astype(np.float32)
    uf = np.asarray(u).reshape(-1).astype(np.float32)
    halves = []
    for h in range(2):
        c0 = h * C_LOC
        halves.append(_derived(wf[c0:c0 + C_LOC], uf[c0:c0 + C_LOC],
                               scan_np_dt))
    in_maps = []
    for core in range(N_CORES):
        b, h = core // 2, core % 2
        c0 = h * C_LOC
        m = {
            "r": np.ascontiguousarray(
                np.asarray(r)[b, :, c0:c0 + C_LOC]).astype(scan_np_dt),
            "k": np.ascontiguousarray(
                np.asarray(k)[b, :, c0:c0 + C_LOC]).astype(scan_np_dt),
            "v": np.ascontiguousarray(
                np.asarray(v)[b, :, c0:c0 + C_LOC]).astype(scan_np_dt),
        }
        m.update(halves[h])
        in_maps.append(m)
    return in_maps


def run(r, k, v, w, u, trace=False, **trace_kwargs):
    """Run on the 8 NeuronCores; returns (y_full, BassKernelResults)."""
    from concourse.bass_utils import run_bass_kernel_spmd

    nc = _get_nc()
    in_maps = _make_in_maps(r, k, v, w, u)
    res = run_bass_kernel_spmd(nc, in_maps, list(range(N_CORES)),
                               trace=trace, **trace_kwargs)
    y = np.empty((B, T, C), np.float32)
    for core in range(N_CORES):
        b, h = core // 2, core % 2
        y[b, :, h * C_LOC:(h + 1) * C_LOC] = \
            res.results[core]["y"].T.astype(np.float32)
    return y, res


def kernel(r, k, v, w, u):
    y, _ = run(r, k, v, w, u)
    return y
